# revision 5
# baseline (speedup 1.0000x reference)
"""Trainium2 Bass kernel for nn_CausalAttention (GNN message passing).

Math (reference):
    pairs[e] = [img[:, src[e]] ; text[:, tgt[e]]]          # B == H == 128
    a[e]     = sigmoid(w2 . relu(W1 @ pairs[e] + b1) + b2) # per-edge gate
    att_img[b, i] = sum_{e: src[e]=i} a[e] * text[b, tgt[e]]
    att_txt[b, t] = sum_{e: tgt[e]=t} a[e] * img[b, src[e]]

Architecture (8 cores, output-column sharding, scatter-free):
  Core c owns att_img[:, 128c:128c+128] (edges with src in that range) and
  att_txt[:, 128c:128c+128] (edges with tgt in that range). Each core:
    1. builds U.T = (W1_img @ img).T and V.T = (W1_txt @ text).T as fp16
       row tables in DRAM (once),
    2. per edge set: dma_gather's U/V rows *transposed* (fp16) so the MLP
       needs only two identity matmuls + relu + per-block dot with w2,
    3. gathers the f32 feature rows for the output side,
    4. builds a scaled one-hot oh[e, iloc] = a[e] * (loc[e] == iloc) in one
       DVE op per 128-edge block, and accumulates
       psum[b, iloc] += g_blk.T @ oh on the PE — the core's exact output
       column slice. No scatter-add (its duplicate-index RMW races on HW),
       no host reduction: the host just concatenates the 8 column slices.
"""

import sys

for _p in ("/opt/trn_rl_repo", "/root/.axon_site/_ro/trn_rl_repo"):
    if _p not in sys.path:
        sys.path.insert(0, _p)

import numpy as np

import concourse.bass as bass
import concourse.tile as tile
from concourse import bacc, mybir
from concourse.masks import make_identity

P = 128          # partitions == batch == hidden
DIM = 1024       # img/text feature count
E = 32768        # edges
NCORES = 8
EC = 4608        # per-pipeline edge capacity (36 blocks; counts ~4096±63)
NBLK = EC // P   # 36
NST = 2          # gather stages (pipelining)
SE = EC // NST   # 2304 edges per stage
NBH = SE // P    # 18 blocks per stage

F32 = mybir.dt.float32
F16 = mybir.dt.float16
I16 = mybir.dt.int16

# chunk layout within a stage (offsets/widths in edges, <=512 for one bank)
CHUNKS = [(0, 512), (512, 512), (1024, 512), (1536, 512), (2048, 256)]
assert sum(w for _, w in CHUNKS) == SE


def _build_program():
    nc = bacc.Bacc(None, target_bir_lowering=False, debug=False)

    img = nc.dram_tensor("img", [P, DIM], F32, kind="ExternalInput")
    txt = nc.dram_tensor("txt", [P, DIM], F32, kind="ExternalInput")
    img_t = nc.dram_tensor("img_t", [DIM, P], F32, kind="ExternalInput")
    txt_t = nc.dram_tensor("txt_t", [DIM, P], F32, kind="ExternalInput")
    w1t_img = nc.dram_tensor("w1t_img", [P, P], F32, kind="ExternalInput")
    w1t_txt = nc.dram_tensor("w1t_txt", [P, P], F32, kind="ExternalInput")
    b1_d = nc.dram_tensor("b1_d", [P, 1], F32, kind="ExternalInput")
    w2_d = nc.dram_tensor("w2_d", [P, 1], F32, kind="ExternalInput")
    b2_d = nc.dram_tensor("b2_d", [P, 1], F32, kind="ExternalInput")
    # per-pipeline index/metadata arrays (i = img side, t = txt side)
    pin = {}
    for s in ("i", "t"):
        pin[s] = dict(
            u16=nc.dram_tensor(f"{s}_u16", [P, EC // 16], I16, kind="ExternalInput"),
            v16=nc.dram_tensor(f"{s}_v16", [P, EC // 16], I16, kind="ExternalInput"),
            f16=nc.dram_tensor(f"{s}_f16", [P, EC // 16], I16, kind="ExternalInput"),
            loc=nc.dram_tensor(f"{s}_loc", [P, NBLK], F32, kind="ExternalInput"),
        )
    out_img = nc.dram_tensor("out_img", [P, P], F32, kind="ExternalOutput")
    out_txt = nc.dram_tensor("out_txt", [P, P], F32, kind="ExternalOutput")

    u_t = nc.dram_tensor("u_t", [DIM, 2, P], F16, kind="Internal")
    v_t = nc.dram_tensor("v_t", [DIM, 2, P], F16, kind="Internal")

    with tile.TileContext(nc) as tc:
        with (
            tc.tile_pool(name="const", bufs=1) as cp,
            tc.tile_pool(name="gath", bufs=2) as gp,
            tc.tile_pool(name="work", bufs=2) as wp,
            tc.tile_pool(name="psH", bufs=2, space="PSUM") as psH,
            tc.tile_pool(name="psA", bufs=2, space="PSUM") as psA,
            tc.tile_pool(name="psACC", bufs=1, space="PSUM") as psACC,
        ):
            # ---- constants ----
            w1i_s = cp.tile([P, P], F32)
            w1x_s = cp.tile([P, P], F32)
            b1_s = cp.tile([P, 1], F32)
            w2_s = cp.tile([P, 1], F32)
            b2_s = cp.tile([P, 1], F32)
            ident = cp.tile([P, P], F16)
            iota_i = cp.tile([P, P], mybir.dt.int32)
            iota_f = cp.tile([P, P], F32)
            img_s = cp.tile([P, DIM], F32)
            txt_s = cp.tile([P, DIM], F32)

            nc.sync.dma_start(w1i_s[:], w1t_img[:])
            nc.sync.dma_start(w1x_s[:], w1t_txt[:])
            nc.sync.dma_start(b1_s[:], b1_d[:])
            nc.sync.dma_start(w2_s[:], w2_d[:])
            nc.sync.dma_start(b2_s[:], b2_d[:])
            nc.sync.dma_start(img_s[:], img[:])
            nc.sync.dma_start(txt_s[:], txt[:])
            make_identity(nc, ident[:])
            nc.gpsimd.iota(iota_i[:], pattern=[[1, P]], base=0, channel_multiplier=0)
            nc.vector.tensor_copy(iota_f[:], iota_i[:])

            idx_s = {}
            loc_s = {}
            for s in ("i", "t"):
                for k in ("u16", "v16", "f16"):
                    t_ = cp.tile([P, EC // 16], I16, tag=f"{s}{k}")
                    nc.sync.dma_start(t_[:], pin[s][k][:])
                    idx_s[(s, k)] = t_
                t_ = cp.tile([P, NBLK], F32, tag=f"{s}loc")
                nc.sync.dma_start(t_[:], pin[s]["loc"][:])
                loc_s[s] = t_

            # ---- build U.T / V.T fp16 tables in DRAM ----
            for feat, w1t, dst in ((img_s, w1i_s, u_t), (txt_s, w1x_s, v_t)):
                sb_hi = wp.tile([P, 8, P], F16, tag="uvhi")
                sb_lo = wp.tile([P, 8, P], F16, tag="uvlo")
                for blk in range(8):
                    ps = psH.tile([P, 512], F32, tag="h_ps")
                    nc.tensor.matmul(
                        ps[:, :P], feat[:, blk * P : (blk + 1) * P], w1t[:],
                        start=True, stop=True,
                    )
                    nc.scalar.copy(sb_hi[:, blk, :], ps[:, :P])
                    nc.vector.tensor_tensor(
                        out=sb_lo[:, blk, :], in0=ps[:, :P], in1=sb_hi[:, blk, :],
                        op=mybir.AluOpType.subtract,
                    )
                nc.sync.dma_start(
                    dst[:, 0, :].rearrange("(a p) f -> p a f", p=P), sb_hi[:]
                )
                nc.sync.dma_start(
                    dst[:, 1, :].rearrange("(a p) f -> p a f", p=P), sb_lo[:]
                )

            # ---- the two pipelines ----
            for side, ftab, out_d in (("i", txt_t, out_img), ("t", img_t, out_txt)):
                acc = psACC.tile([P, P], F32, tag=f"acc{side}")
                for st in range(NST):
                    i0 = st * (SE // 16)
                    gU = gp.tile([P, 2, SE], F16, tag=f"gU")
                    gV = gp.tile([P, 2, SE], F16, tag=f"gV")
                    gF = gp.tile([P, NBH, P], F32, tag=f"gF")
                    nc.gpsimd.dma_gather(
                        out_ap=gU[:], in_ap=u_t[:].rearrange("d a p -> d (a p)"),
                        idxs_ap=idx_s[(side, "u16")][:, i0 : i0 + SE // 16],
                        num_idxs=SE, num_idxs_reg=SE, elem_size=2 * P,
                        transpose=True, single_packet=False,
                    )
                    nc.gpsimd.dma_gather(
                        out_ap=gV[:], in_ap=v_t[:].rearrange("d a p -> d (a p)"),
                        idxs_ap=idx_s[(side, "v16")][:, i0 : i0 + SE // 16],
                        num_idxs=SE, num_idxs_reg=SE, elem_size=2 * P,
                        transpose=True, single_packet=False,
                    )
                    nc.gpsimd.dma_gather(
                        out_ap=gF[:], in_ap=ftab[:],
                        idxs_ap=idx_s[(side, "f16")][:, i0 : i0 + SE // 16],
                        num_idxs=SE, num_idxs_reg=SE, elem_size=P,
                        single_packet=False,
                    )

                    a_ps = psA.tile([P, NBH], F32, tag="a_ps")
                    for off, w in CHUNKS:
                        h_ps = psH.tile([P, 512], F32, tag="h_ps")
                        for mi, gsrc in enumerate(
                            (gU[:, 0, :], gU[:, 1, :], gV[:, 0, :], gV[:, 1, :])
                        ):
                            nc.tensor.matmul(
                                h_ps[:, :w], ident[:], gsrc[:, off : off + w],
                                start=(mi == 0), stop=(mi == 3),
                            )
                        h_s = wp.tile([P, 512], F32, tag="h_s")
                        nc.scalar.activation(
                            h_s[:, :w], h_ps[:, :w],
                            mybir.ActivationFunctionType.Relu, bias=b1_s[:],
                        )
                        for j in range(w // P):
                            kb = off // P + j
                            nc.tensor.matmul(
                                a_ps[:, kb : kb + 1],
                                h_s[:, j * P : (j + 1) * P], w2_s[:],
                                start=True, stop=True,
                            )
                    a_s = wp.tile([P, NBH], F32, tag="a_s")
                    nc.scalar.activation(
                        a_s[:], a_ps[:], mybir.ActivationFunctionType.Sigmoid,
                        bias=b2_s[:],
                    )

                    for kb in range(NBH):
                        blk = st * NBH + kb
                        oh = wp.tile([P, P], F32, tag="oh")
                        nc.vector.tensor_scalar(
                            out=oh[:], in0=iota_f[:],
                            scalar1=loc_s[side][:, blk : blk + 1],
                            scalar2=a_s[:, kb : kb + 1],
                            op0=mybir.AluOpType.is_equal,
                            op1=mybir.AluOpType.mult,
                        )
                        nc.tensor.matmul(
                            acc[:], gF[:, kb, :], oh[:],
                            start=(blk == 0), stop=(blk == NBLK - 1),
                            skip_group_check=True,
                        )
                out_sb = wp.tile([P, P], F32, tag="out_sb")
                nc.vector.tensor_copy(out_sb[:], acc[:])
                nc.sync.dma_start(out_d[:], out_sb[:])

    nc.compile()
    return nc


_PROGRAM = None


def _get_program():
    global _PROGRAM
    if _PROGRAM is None:
        _PROGRAM = _build_program()
    return _PROGRAM


def _wrap16(v: np.ndarray) -> np.ndarray:
    """int16 index layout for dma_gather: idx i at [i % 16, i // 16],
    replicated across the 8 GPSIMD cores (partitions 16..127)."""
    w = v.astype(np.int16).reshape(-1, 16).T
    return np.ascontiguousarray(np.tile(w, (8, 1)))


def _pipe_arrays(key_vals, src_v, tgt_v, base):
    """Build one pipeline's padded index/loc arrays.

    key_vals: the bucketing key values of this pipeline's edges (src or tgt);
    src_v/tgt_v: the edges' src/tgt; base: 128*core.
    """
    n = len(key_vals)
    assert n <= EC, f"bucket overflow: {n} > {EC}"
    u = np.zeros(EC, np.int16)
    v = np.zeros(EC, np.int16)
    f = np.zeros(EC, np.int16)
    loc = np.full(EC, -1.0, np.float32)
    u[:n] = src_v
    v[:n] = tgt_v
    loc[:n] = key_vals - base
    return u, v, loc


def _make_in_maps(img_features, text_features, src, tgt, W1, b1, w2, b2):
    img = np.ascontiguousarray(img_features.astype(np.float32))
    txt = np.ascontiguousarray(text_features.astype(np.float32))
    imgT = np.ascontiguousarray(img.T)
    txtT = np.ascontiguousarray(txt.T)
    w1t_img = np.ascontiguousarray(W1[:, :P].T.astype(np.float32))
    w1t_txt = np.ascontiguousarray(W1[:, P:].T.astype(np.float32))
    b1c = np.ascontiguousarray(b1.astype(np.float32).reshape(P, 1))
    w2c = np.ascontiguousarray(w2.astype(np.float32).reshape(P, 1))
    b2c = np.full((P, 1), np.float32(b2), dtype=np.float32)
    src = np.asarray(src).astype(np.int64)
    tgt = np.asarray(tgt).astype(np.int64)

    in_maps = []
    for c in range(NCORES):
        base = c * P
        m = {
            "img": img, "txt": txt, "img_t": imgT, "txt_t": txtT,
            "w1t_img": w1t_img, "w1t_txt": w1t_txt,
            "b1_d": b1c, "w2_d": w2c, "b2_d": b2c,
        }
        for s, key in (("i", src), ("t", tgt)):
            sel = (key >= base) & (key < base + P)
            sv, tv, kv = src[sel], tgt[sel], key[sel]
            n = len(kv)
            assert n <= EC, f"bucket overflow core {c}: {n} > {EC}"
            u = np.zeros(EC, np.int64)
            v = np.zeros(EC, np.int64)
            f = np.zeros(EC, np.int64)
            loc = np.full(EC, -1.0, np.float32)
            u[:n], v[:n], f[:n] = sv, tv, (tv if s == "i" else sv)
            loc[:n] = kv - base
            m[f"{s}_u16"] = _wrap16(u)
            m[f"{s}_v16"] = _wrap16(v)
            m[f"{s}_f16"] = _wrap16(f)
            m[f"{s}_loc"] = np.ascontiguousarray(
                loc.reshape(NBLK, P).T.astype(np.float32)
            )
        in_maps.append(m)
    return in_maps


def _run(inputs, trace=False):
    from concourse.bass_utils import run_bass_kernel_spmd

    nc = _get_program()
    in_maps = _make_in_maps(**inputs)
    res = run_bass_kernel_spmd(
        nc, in_maps, core_ids=list(range(NCORES)), trace=trace
    )
    att_img = np.concatenate([r["out_img"] for r in res.results], axis=1)
    att_txt = np.concatenate([r["out_txt"] for r in res.results], axis=1)
    return (np.ascontiguousarray(att_img), np.ascontiguousarray(att_txt)), res


def kernel(**inputs):
    out, _ = _run(inputs, trace=False)
    return out


# revision 8
# speedup vs baseline: 2.1487x; 2.1487x over previous
"""Trainium2 Bass kernel for nn_CausalAttention (GNN message passing).

Math (reference):
    pairs[e] = [img[:, src[e]] ; text[:, tgt[e]]]          # B == H == 128
    a[e]     = sigmoid(w2 . relu(W1 @ pairs[e] + b1) + b2) # per-edge gate
    att_img[b, i] = sum_{e: src[e]=i} a[e] * text[b, tgt[e]]
    att_txt[b, t] = sum_{e: tgt[e]=t} a[e] * img[b, src[e]]

Architecture: output-column sharding, fully on-chip (no dma_gather /
dma_scatter_add — the former is descriptor-generation-bound on the Q7,
the latter races on duplicate indices on HW).
Core c owns att_img[:, Wc] and att_txt[:, Wc], Wc = [128c, 128c+128).
For the img pipe (txt pipe symmetric, roles swapped):
  - edges with src in Wc, bucketed by w = tgt >> 7 (8 fixed-capacity
    buckets of 5 blocks of 128 edge slots; unused slots are dummies).
  - tables in SBUF: txtT8[lo, w, b] = text[b, 128w+lo],
    V8[lo, w, h] = (W1_txt @ text).T likewise, U_winT[loc, h] for Wc.
  - per bucket: transposed one-hot masks from host-replicated key rows
      ohKT[loc, e] = (srcloc[e] == loc),  ohLT[lo, e] = (tgtlo[e] == lo)
    h = relu(U_winT.T @ ohKT + V8[w].T @ ohLT + b1)   (PE matmuls)
    a = sigmoid(h.T @ w2 + b2)                        (per-block N=1 mm)
  - per block: M_w[lo, loc] += ohlo.T @ (a * ohK)     (PE, PSUM accum)
  - tail: att[:, loc] = sum_w txtT8[w].T @ M_w        (8 matmuls)
Host just concatenates the 8 column slices. Everything f32.
"""

import sys

for _p in ("/opt/trn_rl_repo", "/root/.axon_site/_ro/trn_rl_repo"):
    if _p not in sys.path:
        sys.path.insert(0, _p)

import numpy as np

import concourse.bass as bass
import concourse.tile as tile
from concourse import bacc, mybir

P = 128
DIM = 1024
E = 32768
NCORES = 8
NW = 8            # hi buckets
BPW = 5           # blocks per bucket (capacity 640 vs mean 512, +6 sigma)
NBLK = NW * BPW   # 40
EC = NBLK * P     # 5120 edge slots per pipeline
BW = BPW * P      # 640 edges per bucket

F32 = mybir.dt.float32
I8 = mybir.dt.int8

IS_EQ = mybir.AluOpType.is_equal
MULT = mybir.AluOpType.mult


def _build_program():
    nc = bacc.Bacc(None, target_bir_lowering=False, debug=False)

    img = nc.dram_tensor("img", [P, DIM], F32, kind="ExternalInput")
    txt = nc.dram_tensor("txt", [P, DIM], F32, kind="ExternalInput")
    img_t = nc.dram_tensor("img_t", [DIM, P], F32, kind="ExternalInput")
    txt_t = nc.dram_tensor("txt_t", [DIM, P], F32, kind="ExternalInput")
    img_win = nc.dram_tensor("img_win", [P, P], F32, kind="ExternalInput")
    txt_win = nc.dram_tensor("txt_win", [P, P], F32, kind="ExternalInput")
    w1t_img = nc.dram_tensor("w1t_img", [P, P], F32, kind="ExternalInput")
    w1t_txt = nc.dram_tensor("w1t_txt", [P, P], F32, kind="ExternalInput")
    b1_d = nc.dram_tensor("b1_d", [P, 1], F32, kind="ExternalInput")
    w2_d = nc.dram_tensor("w2_d", [P, 1], F32, kind="ExternalInput")
    b2_d = nc.dram_tensor("b2_d", [P, 1], F32, kind="ExternalInput")
    pin = {}
    for s in ("i", "t"):
        pin[s] = dict(
            repk=nc.dram_tensor(f"{s}_repk", [P, EC], I8, kind="ExternalInput"),
            repl=nc.dram_tensor(f"{s}_repl", [P, EC], I8, kind="ExternalInput"),
            loc8=nc.dram_tensor(f"{s}_loc8", [P, NBLK], F32, kind="ExternalInput"),
            lo8=nc.dram_tensor(f"{s}_lo8", [P, NBLK], F32, kind="ExternalInput"),
        )
    out_img = nc.dram_tensor("out_img", [P, P], F32, kind="ExternalOutput")
    out_txt = nc.dram_tensor("out_txt", [P, P], F32, kind="ExternalOutput")

    with tile.TileContext(nc) as tc:
        with (
            tc.tile_pool(name="const", bufs=1) as cp,
            tc.tile_pool(name="work", bufs=3) as wp,
            tc.tile_pool(name="psH", bufs=2, space="PSUM") as psH,
            tc.tile_pool(name="psM", bufs=1, space="PSUM") as psM,
            tc.tile_pool(name="psS", bufs=1, space="PSUM") as psS,
        ):
            w1i_s = cp.tile([P, P], F32)
            w1x_s = cp.tile([P, P], F32)
            b1_s = cp.tile([P, 1], F32)
            w2_s = cp.tile([P, 1], F32)
            b2_s = cp.tile([P, 1], F32)
            iota_f = cp.tile([P, P], F32)
            iota_i = cp.tile([P, P], mybir.dt.int32)
            iotap_i = cp.tile([P, 1], mybir.dt.int32)
            iotapf = cp.tile([P, 1], F32)
            imgw_s = cp.tile([P, P], F32)
            txtw_s = cp.tile([P, P], F32)
            img_s = cp.tile([P, DIM], F32)
            txt_s = cp.tile([P, DIM], F32)
            txtT8 = cp.tile([P, NW, P], F32)
            imgT8 = cp.tile([P, NW, P], F32)
            U8 = cp.tile([P, NW, P], F32)
            V8 = cp.tile([P, NW, P], F32)
            UwinT = cp.tile([P, P], F32)
            VwinT = cp.tile([P, P], F32)

            nc.sync.dma_start(w1i_s[:], w1t_img[:])
            nc.sync.dma_start(w1x_s[:], w1t_txt[:])
            nc.sync.dma_start(b1_s[:], b1_d[:])
            nc.sync.dma_start(w2_s[:], w2_d[:])
            nc.sync.dma_start(b2_s[:], b2_d[:])
            nc.sync.dma_start(imgw_s[:], img_win[:])
            nc.sync.dma_start(txtw_s[:], txt_win[:])
            nc.sync.dma_start(img_s[:], img[:])
            nc.sync.dma_start(txt_s[:], txt[:])
            nc.sync.dma_start(
                txtT8[:], txt_t[:].rearrange("(w lo) b -> lo w b", lo=P)
            )
            nc.sync.dma_start(
                imgT8[:], img_t[:].rearrange("(w lo) b -> lo w b", lo=P)
            )
            nc.gpsimd.iota(iota_i[:], pattern=[[1, P]], base=0, channel_multiplier=0)
            nc.vector.tensor_copy(iota_f[:], iota_i[:])
            nc.gpsimd.iota(iotap_i[:], pattern=[[0, 1]], base=0, channel_multiplier=1)
            nc.vector.tensor_copy(iotapf[:], iotap_i[:])

            rep_s = {}
            meta_s = {}
            for s in ("i", "t"):
                for k in ("repk", "repl"):
                    t_ = cp.tile([P, EC], I8, tag=f"{s}{k}")
                    nc.sync.dma_start(t_[:], pin[s][k][:])
                    rep_s[(s, k)] = t_
                for k in ("loc8", "lo8"):
                    t_ = cp.tile([P, NBLK], F32, tag=f"{s}{k}")
                    nc.sync.dma_start(t_[:], pin[s][k][:])
                    meta_s[(s, k)] = t_

            # U8[lo, w, h] = (W1_img @ img).T rows; UwinT likewise for Wc
            for w in range(NW):
                ps = psH.tile([P, BW], F32, tag="h_ps")
                nc.tensor.matmul(
                    ps[:, :P], img_s[:, w * P : (w + 1) * P], w1i_s[:],
                    start=True, stop=True,
                )
                nc.vector.tensor_copy(U8[:, w, :], ps[:, :P])
                ps2 = psH.tile([P, BW], F32, tag="h_ps")
                nc.tensor.matmul(
                    ps2[:, :P], txt_s[:, w * P : (w + 1) * P], w1x_s[:],
                    start=True, stop=True,
                )
                nc.vector.tensor_copy(V8[:, w, :], ps2[:, :P])
            ps = psH.tile([P, BW], F32, tag="h_ps")
            nc.tensor.matmul(ps[:, :P], imgw_s[:], w1i_s[:], start=True, stop=True)
            nc.vector.tensor_copy(UwinT[:], ps[:, :P])
            ps = psH.tile([P, BW], F32, tag="h_ps")
            nc.tensor.matmul(ps[:, :P], txtw_s[:], w1x_s[:], start=True, stop=True)
            nc.vector.tensor_copy(VwinT[:], ps[:, :P])

            for side, arbT8, arbW8, winT, out_d in (
                ("i", txtT8, V8, UwinT, out_img),
                ("t", imgT8, U8, VwinT, out_txt),
            ):
                repk = rep_s[(side, "repk")]
                repl = rep_s[(side, "repl")]
                loc8 = meta_s[(side, "loc8")]
                lo8 = meta_s[(side, "lo8")]
                m_ps0 = psM.tile([P, 4 * P], F32, tag="m0")
                m_ps1 = psM.tile([P, 4 * P], F32, tag="m1")
                m_ps = [m_ps0, m_ps1]
                acc = psS.tile([P, P], F32, tag="acc")
                a_ps = psS.tile([P, NBLK], F32, tag="a_ps")

                # ---- phase A: per-edge gate a ----
                for w in range(NW):
                    e0 = w * BW
                    ohKT = wp.tile([P, BW], F32, tag="ohKT")
                    ohLT = wp.tile([P, BW], F32, tag="ohLT")
                    nc.vector.tensor_scalar(
                        out=ohKT[:], in0=repk[:, e0 : e0 + BW],
                        scalar1=iotapf[:], scalar2=None, op0=IS_EQ,
                    )
                    nc.vector.tensor_scalar(
                        out=ohLT[:], in0=repl[:, e0 : e0 + BW],
                        scalar1=iotapf[:], scalar2=None, op0=IS_EQ,
                    )
                    h_ps = psH.tile([P, BW], F32, tag="h_ps")
                    for o, n in ((0, 512), (512, P)):
                        nc.tensor.matmul(
                            h_ps[:, o : o + n], winT[:], ohKT[:, o : o + n],
                            start=True, stop=False,
                        )
                    for o, n in ((0, 512), (512, P)):
                        nc.tensor.matmul(
                            h_ps[:, o : o + n], arbW8[:, w, :], ohLT[:, o : o + n],
                            start=False, stop=True,
                        )
                    h_s = wp.tile([P, BW], F32, tag="h_s")
                    nc.scalar.activation(
                        h_s[:], h_ps[:], mybir.ActivationFunctionType.Relu,
                        bias=b1_s[:],
                    )
                    for j in range(BPW):
                        b = w * BPW + j
                        nc.tensor.matmul(
                            a_ps[:, b : b + 1], h_s[:, j * P : (j + 1) * P],
                            w2_s[:], start=True, stop=True,
                        )
                a_s = wp.tile([P, NBLK], F32, tag="a_s")
                nc.scalar.activation(
                    a_s[:], a_ps[:], mybir.ActivationFunctionType.Sigmoid,
                    bias=b2_s[:],
                )

                # ---- phase B: M_w[lo, loc] += ohlo.T @ (a * ohK) ----
                for b in range(NBLK):
                    w, j = b // BPW, b % BPW
                    ohlo = wp.tile([P, P], F32, tag="ohlo")
                    ohKs = wp.tile([P, P], F32, tag="ohKs")
                    nc.vector.tensor_scalar(
                        out=ohlo[:], in0=iota_f[:],
                        scalar1=lo8[:, b : b + 1], scalar2=None, op0=IS_EQ,
                    )
                    nc.vector.tensor_scalar(
                        out=ohKs[:], in0=iota_f[:],
                        scalar1=loc8[:, b : b + 1], scalar2=a_s[:, b : b + 1],
                        op0=IS_EQ, op1=MULT,
                    )
                    nc.tensor.matmul(
                        m_ps[w // 4][:, (w % 4) * P : (w % 4 + 1) * P],
                        ohlo[:], ohKs[:],
                        start=(j == 0), stop=(j == BPW - 1),
                        skip_group_check=True,
                    )

                # ---- tail: att[:, loc] = sum_w arbT8[w].T @ M_w ----
                for w in range(NW):
                    m_s = wp.tile([P, P], F32, tag="m_s")
                    nc.vector.tensor_copy(
                        m_s[:], m_ps[w // 4][:, (w % 4) * P : (w % 4 + 1) * P]
                    )
                    nc.tensor.matmul(
                        acc[:], arbT8[:, w, :], m_s[:],
                        start=(w == 0), stop=(w == NW - 1),
                        skip_group_check=True,
                    )
                out_sb = wp.tile([P, P], F32, tag="out_sb")
                nc.vector.tensor_copy(out_sb[:], acc[:])
                nc.sync.dma_start(out_d[:], out_sb[:])

    nc.compile()
    return nc


_PROGRAM = None


def _get_program():
    global _PROGRAM
    if _PROGRAM is None:
        _PROGRAM = _build_program()
    return _PROGRAM


def _pipe_arrays(key, arb, base):
    """key: bucketing key values (src for img pipe); arb: the other endpoint.
    Returns repk, repl [P, EC] i8 row-replicated, loc8/lo8 [P, NBLK] f32."""
    kloc = key - base                 # 0..127
    w = arb >> 7                      # bucket
    lo = arb & 127
    slots = np.full(EC, -1, np.int64)  # slot -> edge index or -1
    fill = np.zeros(NW, np.int64)
    order = np.argsort(w, kind="stable")
    for ei in order:
        wb = w[ei]
        assert fill[wb] < BW, f"bucket overflow: {fill[wb]}"
        slots[wb * BW + fill[wb]] = ei
        fill[wb] += 1
    klocs = np.full(EC, -1, np.int64)
    los = np.full(EC, -1, np.int64)
    used = slots >= 0
    klocs[used] = kloc[slots[used]]
    los[used] = lo[slots[used]]
    repk = np.ascontiguousarray(np.tile(klocs.astype(np.int8)[None, :], (P, 1)))
    repl = np.ascontiguousarray(np.tile(los.astype(np.int8)[None, :], (P, 1)))
    # col layout [P, NBLK]: edge slot e at [e % 128, e // 128]
    loc8 = np.ascontiguousarray(klocs.astype(np.float32).reshape(NBLK, P).T)
    lo8 = np.ascontiguousarray(los.astype(np.float32).reshape(NBLK, P).T)
    return repk, repl, loc8, lo8


def _make_in_maps(img_features, text_features, src, tgt, W1, b1, w2, b2):
    img = np.ascontiguousarray(img_features.astype(np.float32))
    txt = np.ascontiguousarray(text_features.astype(np.float32))
    imgT = np.ascontiguousarray(img.T)
    txtT = np.ascontiguousarray(txt.T)
    w1t_img = np.ascontiguousarray(W1[:, :P].T.astype(np.float32))
    w1t_txt = np.ascontiguousarray(W1[:, P:].T.astype(np.float32))
    b1c = np.ascontiguousarray(b1.astype(np.float32).reshape(P, 1))
    w2c = np.ascontiguousarray(w2.astype(np.float32).reshape(P, 1))
    b2c = np.full((P, 1), np.float32(b2), dtype=np.float32)
    src = np.asarray(src).astype(np.int64)
    tgt = np.asarray(tgt).astype(np.int64)

    in_maps = []
    for c in range(NCORES):
        base = c * P
        m = {
            "img": img, "txt": txt, "img_t": imgT, "txt_t": txtT,
            "img_win": np.ascontiguousarray(img[:, base : base + P]),
            "txt_win": np.ascontiguousarray(txt[:, base : base + P]),
            "w1t_img": w1t_img, "w1t_txt": w1t_txt,
            "b1_d": b1c, "w2_d": w2c, "b2_d": b2c,
        }
        for s, key, arb in (("i", src, tgt), ("t", tgt, src)):
            sel = (key >= base) & (key < base + P)
            repk, repl, loc8, lo8 = _pipe_arrays(key[sel], arb[sel], base)
            m[f"{s}_repk"] = repk
            m[f"{s}_repl"] = repl
            m[f"{s}_loc8"] = loc8
            m[f"{s}_lo8"] = lo8
        in_maps.append(m)
    return in_maps


def _run(inputs, trace=False):
    from concourse.bass_utils import run_bass_kernel_spmd

    nc = _get_program()
    in_maps = _make_in_maps(**inputs)
    res = run_bass_kernel_spmd(
        nc, in_maps, core_ids=list(range(NCORES)), trace=trace
    )
    att_img = np.concatenate([r["out_img"] for r in res.results], axis=1)
    att_txt = np.concatenate([r["out_txt"] for r in res.results], axis=1)
    return (np.ascontiguousarray(att_img), np.ascontiguousarray(att_txt)), res


def kernel(**inputs):
    out, _ = _run(inputs, trace=False)
    return out
